# revision 1
# baseline (speedup 1.0000x reference)
"""Trainium2 Bass kernel for nn_Encoder segment-reduce.

Reference computation (per sample b):
    cls = onehot(argmax_k outputs[b])            # [K, HW]
    sizes = cls.sum(HW) + 0.01                   # [K]
    feat_set = feats[b] @ cls.T / sizes          # [F, K]
    out[b] = w_proj @ feat_set + bias            # [E, K]

Kernel strategy (pure data parallel: 1 sample per NeuronCore, 8 cores).

Mixed int8/bf16 feats, multi-engine expansion, fgrp-major stream.

The kernel is jointly limited by (a) the feats HBM stream, (b) the PE pass
over feats (one moving column per f-column per 128-pixel chunk = 65536 PE
cycles ~ 27us, the dataflow floor), and (c) the on-chip int8->bf16
expansion rate.  bf16 feats alone make DMA the bottleneck (17MB ~ 50us);
int8 alone makes the cast engines the bottleneck (DVE ~407ns + ACT ~712ns
+ GpSimd ~2us per [128,512] chunk < the PE's 216ns/chunk appetite), and a
cast-gated PE is fragile: any transient engine slowdown trips the HAM
half-clock spiral below.  So feats ship half the chunks as int8
(host-quantized, scale 127/4.5) and half as bf16 pre-scaled by the same
127/4.5 (so 1/s folds into w_proj once); the bf16 chunks sit at the end of
each f-group, giving the cast engines a catch-up window every quarter and
leaving every cast engine at <50% utilization.  The stream is then purely
DMA-paced (~14MB at ~345GB/s).  Final rel err ~7e-3 vs the 2e-2 gate.

outputs stay f32: a bf16 argmax flips ~141/32K pixels at class-assignment
ties, and one flipped pixel shifts a whole class mean - 0.13 rel err.

Loop order is fgrp-major (f-groups of 512 outer, hw chunks inner) so each
f-group's [21, 512] segment-reduce PSUM tile completes after its quarter of
the stream; its PSUM copy, PE transpose back to f-major, and projection
matmuls are interleaved into the FIRST HALF of the next quarter's stream.
Emission order per engine is chosen so no engine's queue ever waits on a
result produced later than ~1us after its queue position (a queued wait on
a far-future PE result stalls that engine's later casts, starves the PE,
and trips the HAM death spiral below).

Tail algebra: the per-class reciprocal commutes with the f-contraction, so
the projection accumulates raw sums into out^T [21, 256]; one recip
multiply plus one bias add (bias host-prebroadcast to [21, 256]) finish in
two DVE ops, and the store is a contiguous 1KB-per-partition DMA (host
transposes).

The onehot is 4 DVE instructions total: tensor_reduce over [P, t, 21] and
a broadcast is_equal via tensor_tensor, in two pieces so the first 8
chunks' onehot is ready early.

DMA: feats ride the sync HWDGE queue as 0.4-0.9MB sub-blocks (3.5-7KB
contiguous per partition); outputs/wT/bias/the out store ride the scalar
HWDGE queue so they never delay the feats stream.

HAM: the PE clock ramps 1.2->2.4GHz only under sustained load, and a
multi-us PE idle gap mid-kernel triggers a ~10us half-clock window that
slows EVERY engine (casts included) and spirals.  A warmup matmul burst
bridges the initial DMA window, and the schedule keeps PE duty near 100%
once streaming starts.
"""

import numpy as np

import concourse.bacc as bacc
import concourse.bass as bass
import concourse.mybir as mybir
import concourse.tile as tile
from concourse.bass import ds, ts
from concourse.bass_utils import run_bass_kernel_spmd
from concourse.masks import make_identity

# Problem shapes (hardcoded per contract)
B = 8
K = 21
H = 64
W = 64
HW = H * W            # 4096
F = 2048
E = 256
P = 128
FC = F // P           # 16 f-chunks of 128
FG = 4                # f-groups of 512 (psum accumulate tiles)
FGW = F // FG         # 512
N_T = HW // P         # 32 hw chunks
N_CORES = 8

F32 = mybir.dt.float32
BF16 = mybir.dt.bfloat16
I8 = mybir.dt.int8

QCLIP = 4.5
QSCALE = 127.0 / QCLIP

# int8 chunks per fgrp (rest arrive bf16-direct at the fgrp's end).  The
# bf16 tail both cuts cast volume below the DVE+ACT budget and gives the
# cast pipeline a catch-up window at every fgrp boundary.
N_I8_G = [22, 22, 22, 22]


def _mk_pattern(n, na, g_pos):
    """Cast-engine pattern: D=DVE (~407ns), A=ACT (~712ns), G=GpSimd
    (~2us! - only a few, at low-urgency late positions); A spread evenly."""
    s = ["D"] * n
    for p in g_pos:
        s[p] = "G"
    rest = [i for i in range(n) if s[i] == "D"]
    for j in range(na):
        s[rest[(2 * j + 1) * len(rest) // (2 * na)]] = "A"
    return "".join(s)


CAST_PATTERNS = [_mk_pattern(22, 8, (18,))] * 4
N_CAST_BUFS = 3


def build_module(warmup=90, fillers=8):
    nc = bacc.Bacc("TRN2", target_bir_lowering=False, debug=False,
                   enable_partition_id=False)

    # outputs host-transposed to [p, t, k] (pixel-major).
    outputs_d = nc.dram_tensor("outputs_in", [P, N_T, K], F32, kind="ExternalInput")
    # feats per fgrp: int8 chunks t < N_I8_G[g], bf16 (pre-scaled by
    # QSCALE) for the rest; [p, t, fj]
    feats_i8_d = [
        nc.dram_tensor(f"feats_i8_{g}", [P, N_I8_G[g], FGW], I8,
                       kind="ExternalInput")
        for g in range(FG)
    ]
    feats_bf_d = [
        nc.dram_tensor(f"feats_bf_{g}", [P, N_T - N_I8_G[g], FGW], BF16,
                       kind="ExternalInput")
        for g in range(FG)
    ]
    # (w_proj / s).T rearranged [p, fc, e]
    wT_d = nc.dram_tensor("wT_in", [P, FC, E], BF16, kind="ExternalInput")
    # bias pre-broadcast to [k, e] on host
    bias_d = nc.dram_tensor("bias_in", [K, E], F32, kind="ExternalInput")
    # out^T = [k, e] in bf16 (halves the store; host casts back to f32)
    out_d = nc.dram_tensor("out", [K, E], BF16, kind="ExternalOutput")

    with tile.TileContext(nc) as tc:
        with (
            tc.tile_pool(name="consts", bufs=1) as consts,
            tc.tile_pool(name="fbf", bufs=1) as fbf,
            tc.tile_pool(name="small", bufs=4) as small,
            tc.tile_pool(name="ps_fs", bufs=1, space="PSUM") as ps_fs,
            tc.tile_pool(name="ps_out", bufs=1, space="PSUM") as ps_out,
            tc.tile_pool(name="ps_trp", bufs=1, space="PSUM") as ps_trp,
            tc.tile_pool(name="ps_misc", bufs=1, space="PSUM") as ps_misc,
        ):
            # ---- DMAs ------------------------------------------------
            # scalar HWDGE queue: everything except the feats stream.
            # outputs ride the SYNC queue ahead of feats: the scalar
            # queue's first issue is delayed ~2us by its engine preamble,
            # and the onehot (everything's prerequisite) waits on outputs.
            outputs_sb = consts.tile([P, N_T, K], F32)
            nc.sync.dma_start(out=outputs_sb, in_=outputs_d.ap())
            bias_sb = consts.tile([K, E], F32)
            nc.scalar.dma_start(out=bias_sb, in_=bias_d.ap())
            # wT's dma_start is issued later (after fgrp 0's ACT casts) so
            # its 1MB doesn't compete with the feats ramp; it's only needed
            # by the first projection ~25us in.
            wT_sb = consts.tile([P, FC, E], BF16)

            # sync HWDGE queue: the feats stream.  Each fgrp's int8 part
            # streams ahead of its bf16-direct tail (needed later), and
            # fgrp g's bf block is deferred behind fgrp g+1's first int8
            # block to keep the cast engines fed as early as possible.
            feats_i8_sb = [
                consts.tile([P, N_I8_G[g], FGW], I8, name=f"fi8_{g}")
                for g in range(FG)
            ]
            feats_bf_sb = [
                consts.tile([P, N_T - N_I8_G[g], FGW], BF16, name=f"fbfd_{g}")
                for g in range(FG)
            ]
            i8_blocks = {0: [(0, 6), (6, 14), (14, 22)],
                         1: [(0, 11), (11, 22)],
                         2: [(0, 11), (11, 22)],
                         3: [(0, 11), (11, 22)]}

            def dma_i8(g, bi):
                t0, t1 = i8_blocks[g][bi]
                nc.sync.dma_start(
                    out=feats_i8_sb[g][:, ds(t0, t1 - t0)],
                    in_=feats_i8_d[g].ap()[:, ds(t0, t1 - t0)],
                )

            def dma_bf(g):
                # two halves: chunk n_i8 must not wait on the whole tail
                nb = N_T - N_I8_G[g]
                h = nb // 2
                nc.sync.dma_start(out=feats_bf_sb[g][:, ds(0, h)],
                                  in_=feats_bf_d[g].ap()[:, ds(0, h)])
                nc.sync.dma_start(out=feats_bf_sb[g][:, ds(h, nb - h)],
                                  in_=feats_bf_d[g].ap()[:, ds(h, nb - h)])

            for g in range(FG):
                for bi in range(len(i8_blocks[g])):
                    dma_i8(g, bi)
                dma_bf(g)

            # ---- PE warm-up + constants ------------------------------
            warm_w = consts.tile([P, FGW], BF16)
            nc.vector.memset(warm_w, 0.0)
            warm_ps = ps_misc.tile([P, 64], F32, tag="warm")
            # N=512 filler matmuls hold PE duty at 100% while the cast
            # pipeline builds its lead during fgrp 0 (HAM insurance).
            warm_ps512 = ps_misc.tile([64, FGW], F32, tag="warm512")

            def emit_filler():
                nc.tensor.matmul(warm_ps512, lhsT=warm_w[:, 0:64], rhs=warm_w)

            for _ in range(warmup):
                nc.tensor.matmul(warm_ps[0:64, :], lhsT=warm_w[:, 0:64],
                                 rhs=warm_w[:, 0:64])

            # Preload the ACT engine's Copy activation table so the first
            # real cast doesn't eat the ~1.3us table load mid-stream.
            act_warm = small.tile([1, 1], BF16, tag="actw")
            nc.scalar.activation(out=act_warm, in_=warm_w[0:1, 0:1],
                                 func=mybir.ActivationFunctionType.Copy)

            ident = consts.tile([P, P], F32)
            make_identity(nc, ident)
            ident_b = consts.tile([K, K], BF16)
            nc.vector.tensor_copy(ident_b, ident[:K, :K])
            ones_b = consts.tile([P, 2], BF16)
            nc.vector.memset(ones_b, 1.0)

            # ---- onehot (DVE, 4 instructions in 2 pieces) ------------
            oh_all = consts.tile([P, N_T, K], BF16)
            rowmax = consts.tile([P, N_T, 1], F32)

            def emit_onehot(t0, t1):
                n = t1 - t0
                nc.vector.tensor_reduce(
                    rowmax[:, ds(t0, n)], outputs_sb[:, ds(t0, n)],
                    mybir.AxisListType.X, mybir.AluOpType.max,
                )
                nc.vector.tensor_tensor(
                    oh_all[:, ds(t0, n)], outputs_sb[:, ds(t0, n)],
                    rowmax[:, ds(t0, n)].to_broadcast((P, n, K)),
                    mybir.AluOpType.is_equal,
                )

            # ---- stream tiles ----------------------------------------
            # 3 cast-target buffers: the cast engines run up to 2 fgrps
            # ahead of the PE, so a transient DVE slowdown (SBUF contention
            # with in-flight DMA writes runs casts at ~1/5 speed in bursts)
            # never starves the PE.
            fg_bf = [
                fbf.tile([P, max(N_I8_G), FGW], BF16, name=f"fgbf{i}",
                         tag=f"fgbf{i}")
                for i in range(N_CAST_BUFS)
            ]
            fs_ps = [
                ps_fs.tile([K, FGW], F32, name=f"fs{i}", tag=f"fs{i}")
                for i in range(2)
            ]
            fs_sc = consts.tile([K, F], BF16)
            fsT_sb = consts.tile([P, FC, K], BF16)
            sz_ps = ps_misc.tile([K, 2], F32, tag="sz")
            outT_ps = ps_out.tile([K, E], F32)

            def emit_cast(g, t):
                eng = CAST_PATTERNS[g][t]
                bf = fg_bf[g % N_CAST_BUFS]
                if eng == "D":
                    nc.vector.tensor_copy(bf[:, t, :], feats_i8_sb[g][:, t, :])
                elif eng == "G":
                    nc.gpsimd.tensor_copy(bf[:, t, :], feats_i8_sb[g][:, t, :])
                else:
                    nc.scalar.activation(
                        out=bf[:, t, :], in_=feats_i8_sb[g][:, t, :],
                        func=mybir.ActivationFunctionType.Copy,
                    )

            def emit_stream(g, t0, t1, filler_until=-1):
                bf = fg_bf[g % N_CAST_BUFS]
                n_i8 = N_I8_G[g]
                for t in range(t0, t1):
                    rhs = (bf[:, t, :] if t < n_i8
                           else feats_bf_sb[g][:, t - n_i8, :])
                    nc.tensor.matmul(
                        fs_ps[g % 2], lhsT=oh_all[:, t, :], rhs=rhs,
                        start=(t == 0), stop=(t == N_T - 1),
                    )
                    if t < filler_until and t % 2 == 1:
                        emit_filler()

            # PSUM copies must ride DVE or ACT (GpSimd cannot touch PSUM).
            # They wait on PE results, so their queue position tethers that
            # engine's later casts to PE progress.  They ride ACT: the DVE
            # is the engine that suffers multi-us slowdowns under DMA/SBUF
            # contention, so it stays a pure cast queue free to run ahead;
            # ACT's cadence has been rock-stable in every trace.
            def emit_fs_copy(g):
                nc.scalar.activation(
                    out=fs_sc[:, ds(g * FGW, FGW)], in_=fs_ps[g % 2],
                    func=mybir.ActivationFunctionType.Copy,
                )

            def emit_transposes(g):
                for j in range(4):
                    fc = g * 4 + j
                    trp = ps_trp.tile([P, K], BF16, name=f"trp{fc}",
                                      tag=f"trp{'AB'[fc % 2]}")
                    nc.tensor.transpose(trp, fs_sc[:, ts(fc, P)], ident_b)
                    nc.scalar.activation(
                        out=fsT_sb[:, fc, :], in_=trp,
                        func=mybir.ActivationFunctionType.Copy,
                    )

            def emit_projs(g):
                for j in range(4):
                    fc = g * 4 + j
                    nc.tensor.matmul(
                        outT_ps, lhsT=fsT_sb[:, fc, :], rhs=wT_sb[:, fc, :],
                        start=(fc == 0), stop=(fc == FC - 1),
                    )

            # ---- main schedule ---------------------------------------
            # fgrp 0: onehot piece A, early casts, stream starts; the 32
            # sizes matmuls fill the PE while casts get ahead.
            # onehot first on DVE (casts follow); the PE starts streaming
            # as soon as piece A of the onehot plus the first casts exist,
            # with the 32 sizes matmuls as guaranteed-ready filler after
            # the first 8 chunks.
            emit_onehot(0, 8)
            emit_onehot(8, N_T)
            for t in range(N_I8_G[0]):
                emit_cast(0, t)
            nc.scalar.dma_start(out=wT_sb, in_=wT_d.ap())
            emit_stream(0, 0, 8, filler_until=2 * fillers)
            for t in range(N_T):
                nc.tensor.matmul(
                    sz_ps, lhsT=oh_all[:, t, :], rhs=ones_b,
                    start=(t == 0), stop=(t == N_T - 1),
                )
            emit_stream(0, 8, N_T, filler_until=2 * fillers)

            # fgrps 1..3: previous fgrp's copy/transpose/proj interleave
            # into this fgrp's stream; all casts emitted up front so the
            # DVE/ACT queues are pure casts and run ahead to the buffer
            # limit.
            for g in range(1, FG):
                for t in range(N_I8_G[g]):
                    emit_cast(g, t)
                emit_fs_copy(g - 1)
                emit_stream(g, 0, 8)
                emit_transposes(g - 1)
                emit_stream(g, 8, 24)
                emit_projs(g - 1)
                emit_stream(g, 24, N_T)

            emit_fs_copy(FG - 1)
            emit_transposes(FG - 1)
            emit_projs(FG - 1)
            # dep-free keep-warm matmuls: hold the PE clock at 8/8 through
            # the recip/bias/store tail (they cannot delay the tail chain)
            for _ in range(6):
                emit_filler()

            # ---- tail ------------------------------------------------
            sizes_sb = small.tile([K, 1], F32, tag="sizes")
            nc.vector.tensor_scalar_add(sizes_sb, sz_ps[:, 0:1], 0.01)
            recip = small.tile([K, 1], F32, tag="recip")
            nc.vector.reciprocal(recip, sizes_sb)
            out_tmp = consts.tile([K, E], F32)
            nc.vector.tensor_scalar_mul(out_tmp, outT_ps, recip)
            out_sb = consts.tile([K, E], BF16)
            nc.vector.tensor_add(out_sb, out_tmp, bias_sb)
            nc.scalar.dma_start(out=out_d.ap(), in_=out_sb)

    nc.compile()
    return nc


_CACHE = {}


def make_in_maps(outputs, feats, w_proj, b_proj):
    import ml_dtypes

    outputs = np.asarray(outputs, dtype=np.float32)
    # [B, K, H, W] -> per sample [p, t, k] (pixel-major: hw = t*128 + p)
    outputs_t = np.ascontiguousarray(
        outputs.reshape(B, K, N_T, P).transpose(0, 3, 2, 1)
    )
    feats = np.asarray(feats, dtype=np.float32)
    # [B, F, H, W] -> [b, g, fj, t, p]; per fgrp chunks t < N_I8_G[g] int8,
    # the rest bf16*QSCALE
    f5 = feats.reshape(B, FG, FGW, N_T, P)
    feats_i8 = {}
    feats_bf = {}
    for g in range(FG):
        n = N_I8_G[g]
        q = np.clip(np.round(f5[:, g, :, :n] * QSCALE), -127, 127).astype(np.int8)
        feats_i8[g] = np.ascontiguousarray(q.transpose(0, 3, 2, 1))
        fbf = (f5[:, g, :, n:] * QSCALE).astype(ml_dtypes.bfloat16)
        feats_bf[g] = np.ascontiguousarray(fbf.transpose(0, 3, 2, 1))
    wT = np.ascontiguousarray(
        (np.asarray(w_proj, dtype=np.float32).T / QSCALE)
        .reshape(FC, P, E).transpose(1, 0, 2)
        .astype(ml_dtypes.bfloat16)
    )
    bias = np.ascontiguousarray(
        np.broadcast_to(np.asarray(b_proj, dtype=np.float32)[None, :], (K, E))
    )
    maps = []
    for b in range(B):
        m = {"outputs_in": outputs_t[b], "wT_in": wT, "bias_in": bias}
        for g in range(FG):
            m[f"feats_i8_{g}"] = feats_i8[g][b]
            m[f"feats_bf_{g}"] = feats_bf[g][b]
        maps.append(m)
    return maps


def kernel(outputs, feats, w_proj, b_proj, _trace=False, _trace_kwargs=None,
           _build_kwargs=None):
    key = tuple(sorted((_build_kwargs or {}).items()))
    if key not in _CACHE:
        _CACHE[key] = build_module(**(_build_kwargs or {}))
    nc = _CACHE[key]
    in_maps = make_in_maps(outputs, feats, w_proj, b_proj)
    res = run_bass_kernel_spmd(
        nc,
        in_maps,
        core_ids=list(range(N_CORES)),
        trace=_trace,
        **(_trace_kwargs or {}),
    )
    # out is [K, E] bf16 per sample; full output is [B, E, K] f32
    out = np.stack(
        [np.asarray(r["out"]).astype(np.float32).T for r in res.results]
    )
    if _trace:
        _CACHE["last_results"] = res
    return out



# revision 2
# speedup vs baseline: 1.0801x; 1.0801x over previous
"""Trainium2 Bass kernel for nn_Encoder segment-reduce.

Reference computation (per sample b):
    cls = onehot(argmax_k outputs[b])            # [K, HW]
    sizes = cls.sum(HW) + 0.01                   # [K]
    feat_set = feats[b] @ cls.T / sizes          # [F, K]
    out[b] = w_proj @ feat_set + bias            # [E, K]

Kernel strategy (pure data parallel: 1 sample per NeuronCore, 8 cores).

feats ship as fp8 E3M4 (4 mantissa bits) and feed the PE matmul
DIRECTLY: fp8 streams through the systolic array at bf16 speed, so the
whole int8->bf16 cast pipeline of the previous revision (DVE+ACT+GpSimd
at ~66us of combined work) disappears, and the HBM stream drops to
8.4MB (feats) + 1.0MB (wT bf16) + 0.35MB (outputs f32).  E3M4 on unit
gaussian data costs rel err ~1.2e-2 vs the 2e-2 gate (e4m3 fails at
2.3e-2; int8 would need the cast pipeline back).

outputs stay f32: a bf16 argmax flips ~141/32K pixels at class-
assignment ties, and one flipped pixel shifts a whole class mean.

Loop order is fgrp-major (f-groups of 512 outer, hw chunks inner) so
each f-group's [21, 512] segment-reduce PSUM tile completes after its
quarter of the stream; its PSUM copy, PE transpose back to f-major, and
projection matmuls are interleaved into the next quarter's stream.

Tail algebra: the per-class reciprocal commutes with the f-contraction,
so the projection accumulates raw sums into out^T [21, 256]; one recip
multiply plus one bias add finish in two DVE ops.

HAM: the PE clock ramps 1.2->2.4GHz only under ~3.4us of sustained
load.  A warmup matmul burst starting right after the engine preamble
(~5us) bridges the DMA ramp so the stream starts at full clock.
"""

import numpy as np

import concourse.bacc as bacc
import concourse.bass as bass
import concourse.mybir as mybir
import concourse.tile as tile
from concourse.bass import ds, ts
from concourse.bass_utils import run_bass_kernel_spmd
from concourse.masks import make_identity

# Problem shapes (hardcoded per contract)
B = 8
K = 21
H = 64
W = 64
HW = H * W            # 4096
F = 2048
E = 256
P = 128
FC = F // P           # 16 f-chunks of 128
FG = 4                # f-groups of 512 (psum accumulate tiles)
FGW = F // FG         # 512
N_T = HW // P         # 32 hw chunks
N_CORES = 8

F32 = mybir.dt.float32
BF16 = mybir.dt.bfloat16
FP8 = mybir.dt.float8e3


def build_module(warmup=90):
    nc = bacc.Bacc("TRN2", target_bir_lowering=False, debug=False,
                   enable_partition_id=False)

    # outputs host-transposed to [p, t, k] (pixel-major).
    outputs_d = nc.dram_tensor("outputs_in", [P, N_T, K], F32, kind="ExternalInput")
    # feats [p, g, t, fj] in fp8 e3m4
    feats_d = nc.dram_tensor("feats_in", [P, FG, N_T, FGW], FP8,
                             kind="ExternalInput")
    # w_proj.T rearranged [p, fc, e]
    wT_d = nc.dram_tensor("wT_in", [P, FC, E], BF16, kind="ExternalInput")
    # bias pre-broadcast to [k, e] on host
    bias_d = nc.dram_tensor("bias_in", [K, E], F32, kind="ExternalInput")
    # out^T = [k, e] in bf16 (halves the store; host casts back to f32)
    out_d = nc.dram_tensor("out", [K, E], BF16, kind="ExternalOutput")

    with tile.TileContext(nc) as tc:
        with (
            tc.tile_pool(name="consts", bufs=1) as consts,
            tc.tile_pool(name="small", bufs=4) as small,
            tc.tile_pool(name="ps_fs", bufs=1, space="PSUM") as ps_fs,
            tc.tile_pool(name="ps_out", bufs=1, space="PSUM") as ps_out,
            tc.tile_pool(name="ps_trp", bufs=1, space="PSUM") as ps_trp,
            tc.tile_pool(name="ps_misc", bufs=1, space="PSUM") as ps_misc,
        ):
            # ---- DMAs ------------------------------------------------
            # sync HWDGE queue: outputs (the onehot's prerequisite) ahead
            # of the feats stream, in consumption order.
            outputs_sb = consts.tile([P, N_T, K], F32)
            nc.sync.dma_start(out=outputs_sb[:, ds(0, 8)],
                              in_=outputs_d.ap()[:, ds(0, 8)])
            nc.sync.dma_start(out=outputs_sb[:, ds(8, 24)],
                              in_=outputs_d.ap()[:, ds(8, 24)])

            feats_sb = consts.tile([P, FG, N_T, FGW], FP8)
            FB = 8  # hw chunks per dma block

            for g in range(FG):
                for t0 in range(0, N_T, FB):
                    nc.sync.dma_start(
                        out=feats_sb[:, g, ds(t0, FB)],
                        in_=feats_d.ap()[:, g, ds(t0, FB)],
                    )

            # scalar HWDGE queue: bias + wT (wT only needed by the first
            # projection, a quarter into the stream).
            bias_sb = consts.tile([K, E], F32)
            nc.scalar.dma_start(out=bias_sb, in_=bias_d.ap())
            wT_sb = consts.tile([P, FC, E], BF16)
            nc.scalar.dma_start(out=wT_sb, in_=wT_d.ap())

            # ---- PE warm-up + constants ------------------------------
            warm_w = consts.tile([P, 64], BF16)
            nc.gpsimd.memset(warm_w, 0.0)
            warm_ps = ps_misc.tile([P, 64], F32, tag="warm")
            for _ in range(warmup):
                nc.tensor.matmul(warm_ps[0:64, :], lhsT=warm_w,
                                 rhs=warm_w)

            # Preload the ACT engine's Copy activation table so the first
            # real copy doesn't eat the ~1.3us table load mid-stream.
            act_warm = small.tile([1, 1], BF16, tag="actw")
            nc.scalar.activation(out=act_warm, in_=warm_w[0:1, 0:1],
                                 func=mybir.ActivationFunctionType.Copy)

            ident = consts.tile([P, P], F32)
            make_identity(nc, ident)
            ident_b = consts.tile([K, K], BF16)
            nc.vector.tensor_copy(ident_b, ident[:K, :K])
            ones_f8 = consts.tile([P, 2], FP8)
            nc.vector.memset(ones_f8, 1.0)

            # ---- onehot (DVE, 4 instructions in 2 pieces) ------------
            oh_all = consts.tile([P, N_T, K], FP8)
            rowmax = consts.tile([P, N_T, 1], F32)

            def emit_onehot(t0, t1):
                n = t1 - t0
                nc.vector.tensor_reduce(
                    rowmax[:, ds(t0, n)], outputs_sb[:, ds(t0, n)],
                    mybir.AxisListType.X, mybir.AluOpType.max,
                )
                nc.vector.tensor_tensor(
                    oh_all[:, ds(t0, n)], outputs_sb[:, ds(t0, n)],
                    rowmax[:, ds(t0, n)].to_broadcast((P, n, K)),
                    mybir.AluOpType.is_equal,
                )

            # ---- stream tiles ----------------------------------------
            fs_ps = [
                ps_fs.tile([K, FGW], F32, name=f"fs{i}", tag=f"fs{i}")
                for i in range(2)
            ]
            fs_sc = consts.tile([K, F], BF16)
            fsT_sb = consts.tile([P, FC, K], BF16)
            sz_ps = ps_misc.tile([K, 2], F32, tag="sz")
            outT_ps = ps_out.tile([K, E], F32)

            def emit_stream(g, t0, t1):
                for t in range(t0, t1):
                    nc.tensor.matmul(
                        fs_ps[g % 2], lhsT=oh_all[:, t, :],
                        rhs=feats_sb[:, g, t, :],
                        start=(t == 0), stop=(t == N_T - 1),
                    )

            # PSUM copies ride ACT (DVE is the engine that suffers
            # multi-us slowdowns under DMA/SBUF write contention).
            def emit_fs_copy(g):
                nc.scalar.activation(
                    out=fs_sc[:, ds(g * FGW, FGW)], in_=fs_ps[g % 2],
                    func=mybir.ActivationFunctionType.Copy,
                )

            def emit_transposes(g):
                for j in range(4):
                    fc = g * 4 + j
                    trp = ps_trp.tile([P, K], BF16, name=f"trp{fc}",
                                      tag=f"trp{'AB'[fc % 2]}")
                    nc.tensor.transpose(trp, fs_sc[:, ts(fc, P)], ident_b)
                    nc.scalar.activation(
                        out=fsT_sb[:, fc, :], in_=trp,
                        func=mybir.ActivationFunctionType.Copy,
                    )

            def emit_projs(g):
                for j in range(4):
                    fc = g * 4 + j
                    nc.tensor.matmul(
                        outT_ps, lhsT=fsT_sb[:, fc, :], rhs=wT_sb[:, fc, :],
                        start=(fc == 0), stop=(fc == FC - 1),
                    )

            # ---- main schedule ---------------------------------------
            emit_onehot(0, 8)
            emit_onehot(8, N_T)
            emit_stream(0, 0, 8)
            for t in range(N_T):
                nc.tensor.matmul(
                    sz_ps, lhsT=oh_all[:, t, :], rhs=ones_f8,
                    start=(t == 0), stop=(t == N_T - 1),
                )
            emit_stream(0, 8, N_T)

            for g in range(1, FG):
                emit_fs_copy(g - 1)
                emit_stream(g, 0, 8)
                emit_transposes(g - 1)
                emit_stream(g, 8, 24)
                emit_projs(g - 1)
                emit_stream(g, 24, N_T)

            emit_fs_copy(FG - 1)
            emit_transposes(FG - 1)
            emit_projs(FG - 1)

            # ---- tail ------------------------------------------------
            sizes_sb = small.tile([K, 1], F32, tag="sizes")
            nc.vector.tensor_scalar_add(sizes_sb, sz_ps[:, 0:1], 0.01)
            recip = small.tile([K, 1], F32, tag="recip")
            nc.vector.reciprocal(recip, sizes_sb)
            out_tmp = consts.tile([K, E], F32)
            nc.vector.tensor_scalar_mul(out_tmp, outT_ps, recip)
            out_sb = consts.tile([K, E], BF16)
            nc.vector.tensor_add(out_sb, out_tmp, bias_sb)
            nc.scalar.dma_start(out=out_d.ap(), in_=out_sb)

    nc.compile()
    return nc


_CACHE = {}


def make_in_maps(outputs, feats, w_proj, b_proj):
    import ml_dtypes

    outputs = np.asarray(outputs, dtype=np.float32)
    # [B, K, H, W] -> per sample [p, t, k] (pixel-major: hw = t*128 + p)
    outputs_t = np.ascontiguousarray(
        outputs.reshape(B, K, N_T, P).transpose(0, 3, 2, 1)
    )
    feats = np.asarray(feats, dtype=np.float32)
    # [B, F, H, W] -> [b, p, g, t, fj] in fp8 e3m4
    feats_q = np.ascontiguousarray(
        feats.reshape(B, FG, FGW, N_T, P).transpose(0, 4, 1, 3, 2)
    ).astype(ml_dtypes.float8_e3m4)
    wT = np.ascontiguousarray(
        np.asarray(w_proj, dtype=np.float32).T
        .reshape(FC, P, E).transpose(1, 0, 2)
        .astype(ml_dtypes.bfloat16)
    )
    bias = np.ascontiguousarray(
        np.broadcast_to(np.asarray(b_proj, dtype=np.float32)[None, :], (K, E))
    )
    maps = []
    for b in range(B):
        maps.append({
            "outputs_in": outputs_t[b],
            "feats_in": feats_q[b],
            "wT_in": wT,
            "bias_in": bias,
        })
    return maps


def kernel(outputs, feats, w_proj, b_proj, _trace=False, _trace_kwargs=None,
           _build_kwargs=None):
    key = tuple(sorted((_build_kwargs or {}).items()))
    if key not in _CACHE:
        _CACHE[key] = build_module(**(_build_kwargs or {}))
    nc = _CACHE[key]
    in_maps = make_in_maps(outputs, feats, w_proj, b_proj)
    res = run_bass_kernel_spmd(
        nc,
        in_maps,
        core_ids=list(range(N_CORES)),
        trace=_trace,
        **(_trace_kwargs or {}),
    )
    # out is [K, E] bf16 per sample; full output is [B, E, K] f32
    out = np.stack(
        [np.asarray(r["out"]).astype(np.float32).T for r in res.results]
    )
    if _trace:
        _CACHE["last_results"] = res
    return out


# revision 8
# speedup vs baseline: 1.2873x; 1.1919x over previous
"""Trainium2 Bass kernel for nn_Encoder segment-reduce.

Reference computation (per sample b):
    cls = onehot(argmax_k outputs[b])            # [K, HW]
    sizes = cls.sum(HW) + 0.01                   # [K]
    feat_set = feats[b] @ cls.T / sizes          # [F, K]
    out[b] = w_proj @ feat_set + bias            # [E, K]

Kernel strategy (pure data parallel: 1 sample per NeuronCore, 8 cores).

feats ship as fp8 E3M4 (4 mantissa bits) and feed the PE matmul
DIRECTLY: fp8 streams through the systolic array at bf16 speed, so no
cast pipeline exists at all, and the HBM stream drops to 8.4MB (feats)
+ 1.0MB (wT bf16) + 0.35MB (outputs f32).  E3M4 on unit gaussian data
costs rel err ~1.2e-2 vs the 2e-2 gate (e4m3 fails at 2.3e-2).

The segment-reduce matmul only uses 21 of the PE array's 128 output
columns.  The one-hot is zero-padded to 32 and consecutive hw chunks
are issued to the four 32-column array strips via tile_position
(col-tiling): strip MMs overlap, so the stream runs at the LDWEIGHTS
pace (~110ns/chunk) instead of the serial N=512 pace (216ns/chunk).
Each f-group accumulates into a [128, 512] PSUM tile (4 strips of 32
rows); four row-tiled matmuls against identity blocks sum the strips.

outputs stay f32: a bf16 argmax flips ~141/32K pixels at class-
assignment ties, and one flipped pixel shifts a whole class mean.

Loop order is fgrp-major (f-groups of 512 outer, hw chunks inner);
each f-group's strip-sum, PSUM copy, PE transposes back to f-major and
projection matmuls interleave into the next quarter's stream.

Tail: recip multiply + bias add on DVE, store pre-issued on the idle
GpSimd queue so only the transfer latency is exposed.

HAM: the PE clock ramps 1.2->2.4GHz only under ~3.4us of sustained
load.  A warmup matmul burst starting right after the engine preamble
(~5.7us) bridges the DMA ramp so the stream starts at full clock; too
many warmups delay the stream (PE queue is FIFO), so the count is
sized to end right as the first feats block lands.
"""

import numpy as np

import concourse.bacc as bacc
import concourse.bass as bass
import concourse.mybir as mybir
import concourse.tile as tile
from concourse.bass import ds, ts
from concourse.bass_utils import run_bass_kernel_spmd
from concourse.masks import make_identity

# Problem shapes (hardcoded per contract)
B = 8
K = 21
H = 64
W = 64
HW = H * W            # 4096
F = 2048
E = 256
P = 128
FC = F // P           # 16 f-chunks of 128
FG = 4                # f-groups of 512 (psum accumulate tiles)
FGW = F // FG         # 512
N_T = HW // P         # 32 hw chunks
N_CORES = 8
KP = 32               # one-hot padded to 32 classes (zeros 21..31)
NS = 4                # column strips

F32 = mybir.dt.float32
BF16 = mybir.dt.bfloat16
FP8 = mybir.dt.float8e3


def build_module(warmup=45, ns=4, store_q="gpsimd"):
    nc = bacc.Bacc("TRN2", target_bir_lowering=False, debug=False,
                   enable_partition_id=False)

    # outputs host-transposed to [p, t, k] (pixel-major).
    outputs_d = nc.dram_tensor("outputs_in", [P, N_T, K], F32, kind="ExternalInput")
    # feats [p, g, t, fj] in fp8 e3m4
    feats_d = nc.dram_tensor("feats_in", [P, FG, N_T, FGW], FP8,
                             kind="ExternalInput")
    # w_proj.T rearranged [p, fc, e]
    wT_d = nc.dram_tensor("wT_in", [P, FC, E], BF16, kind="ExternalInput")
    # bias pre-broadcast to [k, e] on host
    bias_d = nc.dram_tensor("bias_in", [K, E], F32, kind="ExternalInput")
    # out^T = [k, e] in bf16 (halves the store; host casts back to f32)
    out_d = nc.dram_tensor("out", [K, E], BF16, kind="ExternalOutput")

    with tile.TileContext(nc) as tc:
        with (
            tc.tile_pool(name="consts", bufs=1) as consts,
            tc.tile_pool(name="small", bufs=4) as small,
            tc.tile_pool(name="ps_fs", bufs=1, space="PSUM") as ps_fs,
            tc.tile_pool(name="ps_fs2", bufs=1, space="PSUM") as ps_fs2,
            tc.tile_pool(name="ps_out", bufs=1, space="PSUM") as ps_out,
        ):
            # ---- DMAs ------------------------------------------------
            # sync HWDGE queue: outputs (the onehot's prerequisite) ahead
            # of the feats stream, in consumption order.
            outputs_sb = consts.tile([P, N_T, K], F32)
            nc.sync.dma_start(out=outputs_sb[:, ds(0, 8)],
                              in_=outputs_d.ap()[:, ds(0, 8)])
            nc.sync.dma_start(out=outputs_sb[:, ds(8, 24)],
                              in_=outputs_d.ap()[:, ds(8, 24)])

            feats_sb = consts.tile([P, FG, N_T, FGW], FP8)
            FB = 8  # hw chunks per dma block

            for g in range(FG):
                for t0 in range(0, N_T, FB):
                    nc.sync.dma_start(
                        out=feats_sb[:, g, ds(t0, FB)],
                        in_=feats_d.ap()[:, g, ds(t0, FB)],
                    )

            # scalar HWDGE queue: bias + wT (wT only needed by the first
            # projection, a quarter into the stream).
            bias_sb = consts.tile([K, E], F32)
            nc.scalar.dma_start(out=bias_sb, in_=bias_d.ap())
            wT_sb = consts.tile([P, FC, E], BF16)
            nc.scalar.dma_start(out=wT_sb, in_=wT_d.ap())

            # ---- PE warm-up + constants ------------------------------
            warm_w = consts.tile([P, 64], BF16)
            nc.gpsimd.memset(warm_w, 0.0)
            # one PSUM bank shared (disjoint slices) by the projection
            # accumulator, the sizes accumulator and the warmup target.
            ps_multi = ps_out.tile([P, FGW], F32, tag="multi")
            outT_ps = ps_multi[0:KP, ds(0, E)]
            sz_ps = ps_multi[0:KP, ds(E, 2)]
            warm_ps = ps_multi[0:64, ds(320, 64)]
            for _ in range(warmup):
                nc.tensor.matmul(warm_ps, lhsT=warm_w,
                                 rhs=warm_w)

            # Preload the ACT engine's Copy activation table so the first
            # real copy doesn't eat the ~1.3us table load mid-stream.
            act_warm = small.tile([1, 1], BF16, tag="actw")
            nc.scalar.activation(out=act_warm, in_=warm_w[0:1, 0:1],
                                 func=mybir.ActivationFunctionType.Copy)

            ident = consts.tile([P, P], F32)
            make_identity(nc, ident)
            ident_b = consts.tile([P, P], BF16)
            nc.vector.tensor_copy(ident_b, ident)
            ones_f8 = consts.tile([P, 2], FP8)
            nc.vector.memset(ones_f8, 1.0)

            # stacked identity [32*ns, 32] for the strip sum, built from
            # same-partition copies of the identity block + one transpose.
            wideI = consts.tile([KP, KP * ns], BF16)
            for c in range(ns):
                nc.vector.tensor_copy(wideI[:, ds(KP * c, KP)],
                                      ident_b[0:KP, 0:KP])
            stackI_ps = ps_fs2.tile([KP * ns, KP], BF16, tag="fs20")
            nc.tensor.transpose(stackI_ps, wideI, ident_b[0:KP, 0:KP])
            stackI = consts.tile([KP * ns, KP], BF16)
            nc.vector.tensor_copy(stackI, stackI_ps)

            # ---- onehot (DVE; zero-padded to 32 classes) -------------
            oh_all = consts.tile([P, N_T, KP], FP8)
            nc.vector.memset(oh_all, 0.0)
            rowmax = consts.tile([P, N_T, 1], F32)

            def emit_onehot(t0, t1):
                n = t1 - t0
                nc.vector.tensor_reduce(
                    rowmax[:, ds(t0, n)], outputs_sb[:, ds(t0, n)],
                    mybir.AxisListType.X, mybir.AluOpType.max,
                )
                nc.vector.tensor_tensor(
                    oh_all[:, ds(t0, n), ds(0, K)], outputs_sb[:, ds(t0, n)],
                    rowmax[:, ds(t0, n)].to_broadcast((P, n, K)),
                    mybir.AluOpType.is_equal,
                )

            # ---- stream tiles ----------------------------------------
            fs_ps = [
                ps_fs.tile([P, FGW], F32, name=f"fs{i}", tag=f"fs{i}")
                for i in range(2)
            ]
            fs2_ps = [
                ps_fs2.tile([KP, FGW], F32, name=f"fs2{i}", tag=f"fs2{i}")
                for i in range(2)
            ]
            fs_sbuf = [
                consts.tile([P, FGW], BF16, name=f"fsb{i}")
                for i in range(2)
            ]
            fs_sc = consts.tile([KP, F], BF16)
            fsT_sb = consts.tile([P, FC, KP], BF16)

            def emit_stream(g, t0, t1):
                for t in range(t0, t1):
                    s = t % ns
                    nc.tensor.matmul(
                        fs_ps[g % 2][ds(32 * s, 32), :],
                        lhsT=oh_all[:, t, :],
                        rhs=feats_sb[:, g, t, :],
                        start=(t < ns), stop=(t >= N_T - ns),
                        tile_position=(0, 32 * s) if ns > 1 else None,
                    )

            NSP = KP * ns  # partitions used by the strips

            # strips -> [32, 512] via 4 row-tiled matmuls against the
            # diagonal blocks of the identity (concurrent row groups).
            def emit_strip_sum(g):
                nc.scalar.activation(
                    out=fs_sbuf[g % 2][0:NSP, :], in_=fs_ps[g % 2][0:NSP, :],
                    func=mybir.ActivationFunctionType.Copy,
                )
                nc.tensor.matmul(
                    fs2_ps[g % 2],
                    lhsT=stackI[0:NSP, :],
                    rhs=fs_sbuf[g % 2][0:NSP, :],
                )

            def emit_fs2_copy(g):
                nc.vector.tensor_copy(
                    fs_sc[:, ds(g * FGW, FGW)], fs2_ps[g % 2])

            def emit_transposes(g):
                for j in range(4):
                    fc = g * 4 + j
                    trp = ps_out.tile([P, KP], BF16, name=f"trp{fc}",
                                      tag=f"trp{'AB'[fc % 2]}")
                    nc.tensor.transpose(trp, fs_sc[:, ts(fc, P)],
                                        ident_b[0:KP, 0:KP])
                    nc.scalar.activation(
                        out=fsT_sb[:, fc, :], in_=trp,
                        func=mybir.ActivationFunctionType.Copy,
                    )

            def emit_projs(g):
                for j in range(4):
                    fc = g * 4 + j
                    nc.tensor.matmul(
                        outT_ps, lhsT=fsT_sb[:, fc, :], rhs=wT_sb[:, fc, :],
                        start=(fc == 0), stop=(fc == FC - 1),
                    )

            # ---- main schedule ---------------------------------------
            emit_onehot(0, 8)
            emit_onehot(8, N_T)
            emit_stream(0, 0, 8)
            for t in range(N_T):
                nc.tensor.matmul(
                    sz_ps, lhsT=oh_all[:, t, :], rhs=ones_f8,
                    start=(t == 0), stop=(t == N_T - 1),
                )
            emit_stream(0, 8, N_T)

            for g in range(1, FG):
                emit_stream(g, 0, 8)
                emit_strip_sum(g - 1)
                emit_fs2_copy(g - 1)
                emit_stream(g, 8, 16)
                emit_transposes(g - 1)
                emit_stream(g, 16, 24)
                emit_projs(g - 1)
                emit_stream(g, 24, N_T)

            emit_strip_sum(FG - 1)
            emit_fs2_copy(FG - 1)
            emit_transposes(FG - 1)
            emit_projs(FG - 1)

            # ---- tail ------------------------------------------------
            sizes_sb = small.tile([K, 1], F32, tag="sizes")
            nc.vector.tensor_scalar_add(sizes_sb, sz_ps[0:K, 0:1], 0.01)
            recip = small.tile([K, 1], F32, tag="recip")
            nc.vector.reciprocal(recip, sizes_sb)
            out_tmp = consts.tile([K, E], F32)
            nc.vector.tensor_scalar_mul(out_tmp, outT_ps[0:K, :], recip)
            out_sb = consts.tile([K, E], BF16)
            nc.vector.tensor_add(out_sb, out_tmp, bias_sb)
            # store rides the otherwise-idle gpsimd queue: the descriptor
            # is issued mid-stream and only waits on out_sb's semaphore.
            getattr(nc, store_q).dma_start(out=out_d.ap(), in_=out_sb)

    nc.compile()
    return nc


_CACHE = {}


def make_in_maps(outputs, feats, w_proj, b_proj):
    import ml_dtypes

    outputs = np.asarray(outputs, dtype=np.float32)
    # [B, K, H, W] -> per sample [p, t, k] (pixel-major: hw = t*128 + p)
    outputs_t = np.ascontiguousarray(
        outputs.reshape(B, K, N_T, P).transpose(0, 3, 2, 1)
    )
    feats = np.asarray(feats, dtype=np.float32)
    # [B, F, H, W] -> [b, p, g, t, fj] in fp8 e3m4
    feats_q = np.ascontiguousarray(
        feats.reshape(B, FG, FGW, N_T, P).transpose(0, 4, 1, 3, 2)
    ).astype(ml_dtypes.float8_e3m4)
    wT = np.ascontiguousarray(
        np.asarray(w_proj, dtype=np.float32).T
        .reshape(FC, P, E).transpose(1, 0, 2)
        .astype(ml_dtypes.bfloat16)
    )
    bias = np.ascontiguousarray(
        np.broadcast_to(np.asarray(b_proj, dtype=np.float32)[None, :], (K, E))
    )
    maps = []
    for b in range(B):
        maps.append({
            "outputs_in": outputs_t[b],
            "feats_in": feats_q[b],
            "wT_in": wT,
            "bias_in": bias,
        })
    return maps


def kernel(outputs, feats, w_proj, b_proj, _trace=False, _trace_kwargs=None,
           _build_kwargs=None):
    key = tuple(sorted((_build_kwargs or {}).items()))
    if key not in _CACHE:
        _CACHE[key] = build_module(**(_build_kwargs or {}))
    nc = _CACHE[key]
    in_maps = make_in_maps(outputs, feats, w_proj, b_proj)
    res = run_bass_kernel_spmd(
        nc,
        in_maps,
        core_ids=list(range(N_CORES)),
        trace=_trace,
        **(_trace_kwargs or {}),
    )
    # out is [K, E] bf16 per sample; full output is [B, E, K] f32
    out = np.stack(
        [np.asarray(r["out"]).astype(np.float32).T for r in res.results]
    )
    if _trace:
        _CACHE["last_results"] = res
    return out
